# revision 13
# baseline (speedup 1.0000x reference)
"""Trainium2 Bass kernel for the DBSE sequential-VAE forward pass.

Data-parallel over batch B=128 across 8 NeuronCores (16 sequences/core).
All matmuls run as float32r (TF32) with fp32 PSUM accumulation.

Self-contained: hardcodes shapes; does not read sibling files.
"""

from contextlib import ExitStack

import numpy as np

import concourse.mybir as mybir
import concourse.tile as tile
from concourse import bacc
from concourse.bass_utils import run_bass_kernel_spmd
from concourse.masks import make_identity

f32 = mybir.dt.float32
f32r = mybir.dt.float32r
bf16 = mybir.dt.bfloat16
AF = mybir.ActivationFunctionType

B, T, FEAT, FC, WIN, ZD, FD, HID = 128, 128, 1024, 512, 8, 32, 256, 256
NW = T // WIN          # 16 windows
NCORE = 8
BC = B // NCORE        # 16 sequences per core
RW = BC * NW           # 256 dyn rows per core
BT = BC * T            # 2048 encoder rows per core
G4 = 4 * FC            # 2048 lstm gate width
G3 = 3 * FC            # 1536 gru gate width
DKC = 272              # decoder rhs cols: 256 (b,w) + 16 stat


def _build_program():
    nc = bacc.Bacc("TRN2", target_bir_lowering=False)

    d = {}

    def di(name, shape, dt=f32r):
        d[name] = nc.dram_tensor(name, shape, dt, kind="ExternalInput")

    def do(name, shape, dt=f32):
        d[name] = nc.dram_tensor(name, shape, dt, kind="ExternalOutput")

    di("xT", [128, 8, BT], bf16)            # x transposed, [p, k, (b t)]
    di("encw", [128, 8, FC], bf16)
    di("encb", [128, 4], f32)
    di("gruwih", [128, 4, G3], bf16)
    di("gruwhh", [128, 4, G3])
    di("sfew", [128, 4, FC])
    di("fcatw", [128, 4, FC])         # [fmean_w | flogvar_w]
    di("lstmwih", [128, 4, G4], bf16)
    di("lstmwhh", [128, 4, G4], bf16)
    di("zcatw", [128, 4, 2 * ZD], bf16)     # [zmean | zlogvar]
    di("p1wih", [ZD, 4 * HID])        # gate order (i,f,o,g)
    di("p1whh", [128, 2, 4 * HID])
    di("p2wih", [128, 2, 4 * HID])
    di("p2whh", [128, 2, 4 * HID])
    di("pcatw", [128, 2, 2 * ZD])     # [pmean | plogvar]
    di("deccatw", [128, 3, 2 * FEAT])  # [decm|decl], K padded 288->384

    di("epsf", [BC, FD], f32)
    di("ffT", [128, 4, BC], bf16)
    di("epszT", [ZD, RW], f32)        # cols (b, w)
    di("epsprT", [ZD, RW], f32)       # cols (t, b)
    di("epsdT", [128, 8, DKC], f32)

    do("fml", [BC, 2 * FD])
    do("fpost", [BC, FD])
    do("zml", [2 * ZD, RW])
    do("zpost", [ZD, RW], f32r)
    do("pml", [2 * ZD, RW])
    do("zprior", [ZD, RW])
    do("recon", [128, 8, DKC])

    with tile.TileContext(nc) as tc:
        _emit(nc, tc, d)
    nc.finalize()
    return nc


def _emit(nc, tc, d):
    ctx = ExitStack()
    with ctx:
        const = ctx.enter_context(tc.tile_pool(name="const", bufs=1))
        state = ctx.enter_context(tc.tile_pool(name="state", bufs=1))
        dramp = ctx.enter_context(tc.tile_pool(name="dramp", bufs=1, space="DRAM"))

        ident = const.tile([128, 128], f32)
        make_identity(nc, ident)
        encb_t = const.tile([128, 4], f32)
        nc.sync.dma_start(encb_t[:], d["encb"][:])
        zeros_t = const.tile([128, 1024], f32)
        nc.vector.memset(zeros_t[:], 0.0)

        # PE warmup: dense dummy matmuls to engage HAM while first DMAs land
        with (
            tc.tile_pool(name="warmp", bufs=1) as warmp,
            tc.tile_pool(name="psW", bufs=1, space="PSUM") as psW,
        ):
            warm_rhs = warmp.tile([128, 256], f32r)
            nc.vector.tensor_copy(warm_rhs[:], zeros_t[:, 0:256])
            wps = psW.tile([128, 256], f32, tag="warm", bufs=1)
            for i in range(100):
                nc.tensor.matmul(wps[:], warm_rhs[:, 0:128], warm_rhs[:],
                                 start=(i == 0), stop=(i == 99))

        zpostT = state.tile([ZD, RW], f32r, tag="zpostT")
        fpostT = state.tile([128, 2, BC], f32r, tag="fpostT")
        zpv = zpostT.rearrange("p (b w) -> p b w", b=BC)
        gi_dram = dramp.tile([128, G3], f32)

        # =================== phases A (enc/GRU/f) + B (dyn/z) ===================
        with (
            tc.tile_pool(name="wlstm", bufs=1) as wlstm,
            tc.tile_pool(name="fcpool", bufs=1) as fcpool,
            tc.tile_pool(name="psGRU", bufs=1, space="PSUM") as psGRU,
        ):
            # lstm weight tiles; DMAs emitted after the encoder loop so the
            # encoder inputs win the DMA queues at startup
            lstmwih_t = wlstm.tile([128, 4, G4], bf16)
            lstmwhh_t = wlstm.tile([128, 4, G4], bf16)
            zcatw_t = wlstm.tile([128, 4, 2 * ZD], bf16)

            fc_T = fcpool.tile([128, 4, BT], bf16)   # tanh(enc) feature-major
            fcv = fc_T.rearrange("p k (b t) -> p k b t", b=BC)

            # ---------- encoder + gi precompute (scope frees before GRU) ----------
            with (
                tc.tile_pool(name="wENC", bufs=1) as wENC,
                tc.tile_pool(name="psP1", bufs=1, space="PSUM") as psP1,
            ):
                encw_t = wENC.tile([128, 8, FC], bf16)
                nc.sync.dma_start(encw_t[:], d["encw"][:])
                gruwih_t = wENC.tile([128, 4, G3], bf16)
                ffT_t = wENC.tile([128, 4, BC], bf16)

                for c in range(4):
                    xt = wENC.tile([128, 8, 512], bf16, tag="x", bufs=2)
                    nc.sync.dma_start(
                        xt[:], d["xT"][:, :, c * 512:(c + 1) * 512])
                    for m in range(4):
                        ps = psP1.tile([128, 512], f32, tag="enc", bufs=2)
                        for k in range(8):
                            nc.tensor.matmul(
                                ps[:],
                                encw_t[:, k, m * 128:(m + 1) * 128],
                                xt[:, k, :],
                                start=(k == 0), stop=(k == 7),
                            )
                        nc.scalar.activation(
                            fc_T[:, m, c * 512:(c + 1) * 512], ps[:],
                            AF.Tanh, bias=encb_t[:, m:m + 1],
                        )

                nc.sync.dma_start(gruwih_t[:], d["gruwih"][:])
                nc.sync.dma_start(ffT_t[:], d["ffT"][:])
                nc.sync.dma_start(lstmwih_t[:], d["lstmwih"][:])
                nc.sync.dma_start(lstmwhh_t[:], d["lstmwhh"][:])
                nc.sync.dma_start(zcatw_t[:], d["zcatw"][:])

                # gi = x_win @ gru_wih, rows (t, b); staged via DRAM
                fcwin = wENC.tile([128, 4, 128], bf16)   # cols (t, b)
                nc.vector.tensor_copy(
                    fcwin.rearrange("p k (t b) -> p k t b", t=WIN),
                    fcv[:, :, :, 0:WIN].rearrange("p k b t -> p k t b"),
                )
                for n in range(3):
                    ps = psP1.tile([128, 512], f32, tag="enc", bufs=2)
                    for k in range(4):
                        nc.tensor.matmul(
                            ps[:], fcwin[:, k, :],
                            gruwih_t[:, k, n * 512:(n + 1) * 512],
                            start=(k == 0), stop=(k == 3),
                        )
                    gtmp = wENC.tile([128, 512], f32, tag="gtmp", bufs=2)
                    nc.vector.tensor_copy(gtmp[:], ps[:])
                    nc.sync.dma_start(gi_dram[:, n * 512:(n + 1) * 512], gtmp[:])

                # overwrite fc window-0 cols with ff (dyn branch input)
                nc.vector.tensor_copy(
                    fcv[:, :, :, 0:WIN],
                    ffT_t[:, :, :, None].to_broadcast([128, 4, BC, WIN]),
                )

            # ---------------- GRU + static feature chain + dyn ----------------
            with tc.tile_pool(name="wGRU", bufs=1) as wGRU:
                gruwhh_t = wGRU.tile([128, 4, G3], f32r)
                nc.sync.dma_start(gruwhh_t[:], d["gruwhh"][:])
                sfew_t = wGRU.tile([128, 4, FC], f32r)
                nc.sync.dma_start(sfew_t[:], d["sfew"][:])
                fcatw_t = wGRU.tile([128, 4, FC], f32r)
                nc.sync.dma_start(fcatw_t[:], d["fcatw"][:])
                epsf_t = wGRU.tile([BC, FD], f32)
                nc.sync.dma_start(epsf_t[:], d["epsf"][:])

                hB = wGRU.tile([BC, FC], f32, tag="gruh", bufs=2)
                nc.vector.memset(hB[:], 0.0)
                hT = wGRU.tile([128, 4, BC], f32r, tag="gruhT", bufs=2)
                nc.vector.tensor_copy(hT.rearrange("p k b -> p (k b)"), zeros_t[:, 0:64])
                for t in range(WIN):
                    gi_t = wGRU.tile([BC, G3], f32, tag="git", bufs=1)
                    nc.sync.dma_start(gi_t[:], gi_dram[t * BC:(t + 1) * BC, :])
                    pss = []
                    for n in range(3):
                        ps = psGRU.tile([BC, 512], f32, tag="gh", bufs=2)
                        for k in range(4):
                            nc.tensor.matmul(
                                ps[:], hT[:, k, :],
                                gruwhh_t[:, k, n * 512:(n + 1) * 512],
                                start=(k == 0), stop=(k == 3),
                            )
                        pss.append(ps)
                    prz = wGRU.tile([BC, 1024], f32, tag="prz", bufs=1)
                    nc.vector.tensor_add(prz[:, 0:512], gi_t[:, 0:512], pss[0][:])
                    nc.vector.tensor_add(
                        prz[:, 512:1024], gi_t[:, 512:1024], pss[1][:])
                    srz = wGRU.tile([BC, 1024], f32, tag="srz", bufs=1)
                    nc.scalar.activation(srz[:], prz[:], AF.Sigmoid)
                    rhn = wGRU.tile([BC, 512], f32, tag="rhn", bufs=1)
                    nc.vector.tensor_mul(rhn[:], srz[:, 0:512], pss[2][:])
                    nc.vector.tensor_add(rhn[:], rhn[:], gi_t[:, 1024:1536])
                    n_g = wGRU.tile([BC, 512], f32, tag="ng", bufs=1)
                    nc.scalar.activation(n_g[:], rhn[:], AF.Tanh)
                    dd = wGRU.tile([BC, 512], f32, tag="dd", bufs=1)
                    nc.vector.tensor_sub(dd[:], hB[:], n_g[:])
                    nc.vector.tensor_mul(dd[:], srz[:, 512:1024], dd[:])
                    hB = wGRU.tile([BC, FC], f32, tag="gruh", bufs=2)
                    nc.vector.tensor_add(hB[:], n_g[:], dd[:])
                    pst = psGRU.tile([128, 64], f32, tag="tp", bufs=1)
                    for j in range(4):
                        nc.tensor.transpose(
                            pst[:, j * BC:(j + 1) * BC],
                            hB[:, j * 128:(j + 1) * 128], ident[:BC, :BC],
                        )
                    hT = wGRU.tile([128, 4, BC], f32r, tag="gruhT", bufs=2)
                    nc.vector.tensor_copy(
                        hT[:], pst.rearrange("p (k b) -> p k b", k=4))

                # ---- static feature f, f_mean/f_logvar, f_post ----
                psf = psGRU.tile([BC, 512], f32, tag="gh", bufs=2)
                for k in range(4):
                    nc.tensor.matmul(psf[:], hT[:, k, :], sfew_t[:, k, :],
                                     start=(k == 0), stop=(k == 3))
                fB = wGRU.tile([BC, FC], f32, tag="fB")
                nc.scalar.activation(fB[:], psf[:], AF.Tanh)
                pstf = psGRU.tile([128, 64], f32, tag="tp", bufs=1)
                for j in range(4):
                    nc.tensor.transpose(
                        pstf[:, j * BC:(j + 1) * BC],
                        fB[:, j * 128:(j + 1) * 128], ident[:BC, :BC],
                    )
                fT = wGRU.tile([128, 4, BC], f32r, tag="fT")
                nc.vector.tensor_copy(
                    fT[:], pstf.rearrange("p (k b) -> p k b", k=4))
                psfm = psGRU.tile([BC, 512], f32, tag="gh", bufs=2)
                for k in range(4):
                    nc.tensor.matmul(psfm[:], fT[:, k, :], fcatw_t[:, k, :],
                                     start=(k == 0), stop=(k == 3))
                fml = wGRU.tile([BC, 512], f32, tag="fml")
                nc.vector.tensor_copy(fml[:], psfm[:])
                nc.sync.dma_start(d["fml"][:], fml[:])
                # f_post = mean + epsf * exp(0.5*logvar); exp via sigmoid ratio
                sa = wGRU.tile([BC, FD], f32, tag="sa")
                nc.scalar.activation(sa[:], fml[:, FD:2 * FD], AF.Sigmoid,
                                     scale=0.5)
                sb = wGRU.tile([BC, FD], f32, tag="sb")
                nc.scalar.activation(sb[:], fml[:, FD:2 * FD], AF.Sigmoid,
                                     scale=-0.5)
                nc.vector.reciprocal(sb[:], sb[:])
                nc.vector.tensor_mul(sa[:], sa[:], sb[:])       # exp(0.5 lv)
                nc.vector.tensor_mul(sa[:], sa[:], epsf_t[:])
                fpost = wGRU.tile([BC, FD], f32, tag="fpost")
                nc.vector.tensor_add(fpost[:], fml[:, 0:FD], sa[:])
                nc.sync.dma_start(d["fpost"][:], fpost[:])
                pstp = psGRU.tile([128, 64], f32, tag="tp", bufs=1)
                for j in range(2):
                    nc.tensor.transpose(
                        pstp[:, j * BC:(j + 1) * BC],
                        fpost[:, j * 128:(j + 1) * 128], ident[:BC, :BC],
                    )
                nc.vector.tensor_copy(
                    fpostT[:], pstp[:, 0:32].rearrange("p (k b) -> p k b", k=2))

                # =============== dynamic LSTM (feature-major, fused) ===========
                with (
                    tc.tile_pool(name="dynp", bufs=1) as dynp,
                    tc.tile_pool(name="psDYN", bufs=1, space="PSUM") as psDYN,
                ):
                    hTd = dynp.tile([128, 4, RW], bf16, tag="dynh", bufs=2)
                    nc.vector.tensor_copy(hTd.rearrange("p k n -> p (k n)"), zeros_t[:])
                    cd = dynp.tile([128, 4 * RW], f32, tag="dync", bufs=1)
                    nc.vector.memset(cd[:], 0.0)
                    fcz = fc_T.rearrange("p k (b w t) -> p k b w t", b=BC, w=NW)
                    gnames = ("si", "sf", "tg", "so")

                    for t in range(WIN):
                        newg = []
                        for g, func in enumerate(
                                (AF.Sigmoid, AF.Sigmoid, AF.Tanh, AF.Sigmoid)):
                            ps = psDYN.tile([128, 4 * RW], f32, tag="dyn",
                                            bufs=2)
                            for mi in range(4):
                                m = g * 4 + mi
                                for k in range(4):
                                    nc.tensor.matmul(
                                        ps[:, mi * RW:(mi + 1) * RW],
                                        lstmwih_t[:, k, m * 128:(m + 1) * 128],
                                        fcz[:, k, :, :, t],
                                        start=(k == 0), stop=False,
                                    )
                                for k in range(4):
                                    nc.tensor.matmul(
                                        ps[:, mi * RW:(mi + 1) * RW],
                                        lstmwhh_t[:, k, m * 128:(m + 1) * 128],
                                        hTd[:, k, :],
                                        start=False, stop=(k == 3),
                                    )
                            gt = dynp.tile([128, 4 * RW], f32, tag=gnames[g],
                                           bufs=1)
                            nc.scalar.activation(gt[:], ps[:], func)
                            newg.append(gt)
                        si, sf, tg, so = newg
                        tmp = dynp.tile([128, 4 * RW], f32, tag="dyntmp",
                                        bufs=1)
                        nc.vector.tensor_mul(tmp[:], si[:], tg[:])
                        nc.vector.tensor_mul(cd[:], sf[:], cd[:])
                        nc.vector.tensor_add(cd[:], cd[:], tmp[:])
                        thc = dynp.tile([128, 4 * RW], f32, tag="dyntmp",
                                        bufs=1)
                        nc.scalar.activation(thc[:], cd[:], AF.Tanh)
                        hTd = dynp.tile([128, 4, RW], bf16, tag="dynh", bufs=2)
                        nc.vector.tensor_mul(
                            hTd.rearrange("p k n -> p (k n)"), so[:], thc[:])

                    # ---- z projection + z_post ----
                    psz = psDYN.tile([2 * ZD, RW], f32, tag="z", bufs=1)
                    for k in range(4):
                        nc.tensor.matmul(psz[:], zcatw_t[:, k, :],
                                         hTd[:, k, :],
                                         start=(k == 0), stop=(k == 3))
                    zml = dynp.tile([2 * ZD, RW], f32, tag="si", bufs=1)
                    nc.vector.tensor_copy(zml[:], psz[:])
                    nc.sync.dma_start(d["zml"][:], zml[:])
                    za = dynp.tile([ZD, RW], f32, tag="sf", bufs=1)
                    nc.scalar.activation(za[:], zml[ZD:2 * ZD, :], AF.Sigmoid,
                                         scale=0.5)
                    zb = dynp.tile([ZD, RW], f32, tag="tg", bufs=1)
                    nc.scalar.activation(zb[:], zml[ZD:2 * ZD, :], AF.Sigmoid,
                                         scale=-0.5)
                    nc.vector.reciprocal(zb[:], zb[:])
                    nc.vector.tensor_mul(za[:], za[:], zb[:])
                    epszT_t = dynp.tile([ZD, RW], f32, tag="so", bufs=1)
                    nc.sync.dma_start(epszT_t[:], d["epszT"][:])
                    nc.vector.tensor_mul(za[:], za[:], epszT_t[:])
                    nc.vector.tensor_add(zpostT[:], zml[0:ZD, :], za[:])
                    nc.sync.dma_start(d["zpost"][:], zpostT[:])

        # ============== phase C: prior (2-layer LSTM) + decoder ==============
        with (
            tc.tile_pool(name="wC", bufs=1) as wC,
            tc.tile_pool(name="psPRI", bufs=1, space="PSUM") as psPRI,
        ):
            p1wih_t = wC.tile([ZD, 4 * HID], f32r)
            nc.sync.dma_start(p1wih_t[:], d["p1wih"][:])
            p1whh_t = wC.tile([128, 2, 4 * HID], f32r)
            nc.sync.dma_start(p1whh_t[:], d["p1whh"][:])
            p2wih_t = wC.tile([128, 2, 4 * HID], f32r)
            nc.sync.dma_start(p2wih_t[:], d["p2wih"][:])
            p2whh_t = wC.tile([128, 2, 4 * HID], f32r)
            nc.sync.dma_start(p2whh_t[:], d["p2whh"][:])
            pcatw_t = wC.tile([128, 2, 2 * ZD], f32r)
            nc.sync.dma_start(pcatw_t[:], d["pcatw"][:])
            deccatw_t = wC.tile([128, 3, 2 * FEAT], f32r)
            nc.sync.dma_start(deccatw_t[:], d["deccatw"][:])
            epsprT_t = wC.tile([ZD, RW], f32)
            nc.sync.dma_start(epsprT_t[:], d["epsprT"][:])
            epsdT_t = wC.tile([128, 8, DKC], f32)
            nc.sync.dma_start(epsdT_t[:], d["epsdT"][:])

            # zin: teacher-forced z_post shifted by one window, cols (t, b)
            zinT = wC.tile([ZD, RW], f32r)
            nc.vector.tensor_copy(zinT[:, 0:BC], zeros_t[0:32, 0:BC])
            nc.vector.tensor_copy(
                zinT.rearrange("p (t b) -> p t b", t=NW)[:, 1:NW, :],
                zpv[:, :, 0:NW - 1].rearrange("p b w -> p w b"),
            )

            h1T = wC.tile([128, 2, BC], f32r, tag="h1T", bufs=2)
            nc.vector.tensor_copy(h1T.rearrange("p k b -> p (k b)"), zeros_t[:, 0:32])
            c1 = wC.tile([BC, HID], f32, tag="c1", bufs=2)
            nc.vector.memset(c1[:], 0.0)
            c2 = wC.tile([BC, HID], f32, tag="c2", bufs=2)
            nc.vector.memset(c2[:], 0.0)
            h2all = wC.tile([128, 2, RW], f32r)
            h2z = wC.tile([128, 2, BC], f32r)
            nc.vector.tensor_copy(h2z.rearrange("p k b -> p (k b)"), zeros_t[:, 0:32])

            def lstm_cell(psg, cprev, ctag):
                """gates (i,f,o,g) psum [16,1024] -> (h [16,256], c_new)"""
                # tanh(g) first (ACT), then sig(i,f) so the c-chain starts asap;
                # sig(o) runs on ACT while DVE does the c update.
                tg = wC.tile([BC, HID], f32, tag="tg" + ctag, bufs=2)
                nc.scalar.activation(tg[:], psg[:, 3 * HID:4 * HID], AF.Tanh)
                sif = wC.tile([BC, 2 * HID], f32, tag="sif" + ctag, bufs=2)
                nc.scalar.activation(sif[:], psg[:, 0:2 * HID], AF.Sigmoid)
                so = wC.tile([BC, HID], f32, tag="so" + ctag, bufs=2)
                nc.scalar.activation(so[:], psg[:, 2 * HID:3 * HID], AF.Sigmoid)
                cn = wC.tile([BC, HID], f32, tag=ctag, bufs=2)
                nc.vector.tensor_mul(cn[:], sif[:, HID:2 * HID], cprev[:])
                tmp = wC.tile([BC, HID], f32, tag="tmp" + ctag, bufs=2)
                nc.vector.tensor_mul(tmp[:], sif[:, 0:HID], tg[:])
                nc.vector.tensor_add(cn[:], cn[:], tmp[:])
                th = wC.tile([BC, HID], f32, tag="th" + ctag, bufs=2)
                nc.scalar.activation(th[:], cn[:], AF.Tanh)
                hb = wC.tile([BC, HID], f32, tag="hb" + ctag, bufs=2)
                nc.vector.tensor_mul(hb[:], so[:], th[:])
                return hb, cn

            fillps = psPRI.tile([128, 256], f32, tag="fill", bufs=1)
            nc.tensor.matmul(fillps[:], zpostT[:, 0:128], zpostT[:],
                             start=True, stop=False)
            for i in range(60):
                nc.tensor.matmul(fillps[:], zpostT[:, 0:128], zpostT[:],
                                 start=False, stop=(i == 59))

            for t in range(NW):
                ps1 = psPRI.tile([BC, 4 * HID], f32, tag="pg", bufs=2)
                for n in range(2):
                    nc.tensor.matmul(
                        ps1[:, n * 512:(n + 1) * 512],
                        zinT[:, t * BC:(t + 1) * BC],
                        p1wih_t[:, n * 512:(n + 1) * 512],
                        start=True, stop=False,
                    )
                    for k in range(2):
                        nc.tensor.matmul(
                            ps1[:, n * 512:(n + 1) * 512],
                            h1T[:, k, :], p1whh_t[:, k, n * 512:(n + 1) * 512],
                            start=False, stop=(k == 1),
                        )
                h1b, c1 = lstm_cell(ps1, c1, "c1")
                pst1 = psPRI.tile([128, 32], f32, tag="ptp", bufs=1)
                for j in range(2):
                    nc.tensor.transpose(
                        pst1[:, j * BC:(j + 1) * BC],
                        h1b[:, j * 128:(j + 1) * 128], ident[:BC, :BC],
                    )
                h1T = wC.tile([128, 2, BC], f32r, tag="h1T", bufs=2)
                nc.vector.tensor_copy(
                    h1T[:], pst1.rearrange("p (k b) -> p k b", k=2))

                h2prev = h2z if t == 0 else h2all[:, :, (t - 1) * BC:t * BC]
                ps2 = psPRI.tile([BC, 4 * HID], f32, tag="pg", bufs=2)
                for n in range(2):
                    for k in range(2):
                        nc.tensor.matmul(
                            ps2[:, n * 512:(n + 1) * 512],
                            h1T[:, k, :], p2wih_t[:, k, n * 512:(n + 1) * 512],
                            start=(k == 0), stop=False,
                        )
                    for k in range(2):
                        nc.tensor.matmul(
                            ps2[:, n * 512:(n + 1) * 512],
                            h2prev[:, k, :],
                            p2whh_t[:, k, n * 512:(n + 1) * 512],
                            start=False, stop=(k == 1),
                        )
                h2b, c2 = lstm_cell(ps2, c2, "c2")
                pst2 = psPRI.tile([128, 32], f32, tag="ptp", bufs=1)
                for j in range(2):
                    nc.tensor.transpose(
                        pst2[:, j * BC:(j + 1) * BC],
                        h2b[:, j * 128:(j + 1) * 128], ident[:BC, :BC],
                    )
                nc.vector.tensor_copy(
                    h2all[:, :, t * BC:(t + 1) * BC],
                    pst2.rearrange("p (k b) -> p k b", k=2),
                )

            # pmean/plogvar batched over all steps
            psp = psPRI.tile([2 * ZD, RW], f32, tag="ptp", bufs=1)
            for k in range(2):
                nc.tensor.matmul(psp[:], pcatw_t[:, k, :], h2all[:, k, :],
                                 start=(k == 0), stop=(k == 1))
            pml = wC.tile([2 * ZD, RW], f32)
            nc.vector.tensor_copy(pml[:], psp[:])
            nc.sync.dma_start(d["pml"][:], pml[:])
            pe = wC.tile([ZD, RW], f32)
            nc.scalar.activation(pe[:], pml[ZD:2 * ZD, :], AF.Exp, scale=0.5)
            nc.vector.tensor_mul(pe[:], pe[:], epsprT_t[:])
            nc.vector.tensor_add(pe[:], pml[0:ZD, :], pe[:])
            nc.sync.dma_start(d["zprior"][:], pe[:])

            # ---- decoder ----
            zfT = wC.tile([128, 3, DKC], f32r)
            nc.vector.tensor_copy(zfT[0:32, 0, 0:RW], zpostT[:])
            nc.vector.tensor_copy(zfT[0:32, 0, RW:DKC], zpv[:, :, 0])

            def f_rows(dst, src):
                nc.vector.tensor_copy(
                    dst[:, 0:RW].rearrange("p (b w) -> p b w", b=BC),
                    src[:, :, None].to_broadcast(list(src.shape) + [NW]),
                )
                nc.vector.tensor_copy(dst[:, RW:DKC], src[:])

            # zf row 32+f <- f_post feature f, in 32-partition blocks
            for f0 in range(0, FD, 32):
                r = 32 + f0
                f_rows(zfT[r % 128:r % 128 + 32, r // 128, :],
                       fpostT[f0 % 128:f0 % 128 + 32, f0 // 128, :])

            recon = wC.tile([128, 8, DKC], f32)
            for j in range(8):
                psdm = psPRI.tile([128, DKC], f32, tag="dec", bufs=2)
                psdl = psPRI.tile([128, DKC], f32, tag="dec", bufs=2)
                for ps, m in ((psdm, j), (psdl, j + 8)):
                    for k in range(2):
                        nc.tensor.matmul(
                            ps[:], deccatw_t[:, k, m * 128:(m + 1) * 128],
                            zfT[:, k, :], start=(k == 0), stop=False,
                        )
                    nc.tensor.matmul(
                        ps[:], deccatw_t[0:32, 2, m * 128:(m + 1) * 128],
                        zfT[0:32, 2, :], start=False, stop=True,
                    )
                ee = wC.tile([128, DKC], f32, tag="dece", bufs=2)
                nc.scalar.activation(ee[:], psdl[:], AF.Exp, scale=0.5)
                nc.vector.tensor_mul(ee[:], ee[:], epsdT_t[:, j, :])
                nc.vector.tensor_add(recon[:, j, :], psdm[:], ee[:])
            nc.sync.dma_start(d["recon"][:], recon[:])


_PROG_CACHE = {}


def _get_program():
    if "nc" not in _PROG_CACHE:
        _PROG_CACHE["nc"] = _build_program()
    return _PROG_CACHE["nc"]


def _host_eps():
    if "eps" in _PROG_CACHE:
        return _PROG_CACHE["eps"]
    import jax
    import jax.numpy as jnp
    cpu = jax.local_devices(backend="cpu")[0]
    with jax.default_device(cpu):
        kf, kff, kz, kpr, kdx, kds = jax.random.split(jax.random.key(7), 6)
        eps = {
            "f": np.asarray(jax.random.normal(kf, (B, 1, FD), jnp.float32)),
            "ff": np.asarray(jax.random.normal(kff, (B, FC), jnp.float32)),
            "z": np.asarray(jax.random.normal(kz, (B, NW, ZD), jnp.float32)),
            "pr": np.asarray(jax.random.normal(kpr, (NW, B, ZD), jnp.float32)),
            "dx": np.asarray(jax.random.normal(kdx, (B, NW, FEAT), jnp.float32)),
            "ds": np.asarray(jax.random.normal(kds, (B, 1, FEAT), jnp.float32)),
        }
    _PROG_CACHE["eps"] = eps
    return eps


def _kt(w, kt):
    """[K, N] -> [128, K//128, N] partition-tiled, contiguous."""
    K, N = w.shape
    assert K == kt * 128
    return np.ascontiguousarray(w.reshape(kt, 128, N).transpose(1, 0, 2))


def _permute_ifog(w):
    """LSTM gate columns (i,f,g,o) -> (i,f,o,g). w: [K, 4H]"""
    K, G = w.shape
    H = G // 4
    return np.concatenate(
        [w[:, 0:2 * H], w[:, 3 * H:4 * H], w[:, 2 * H:3 * H]], axis=1)


def kernel(**inputs):
    inp = {k: np.asarray(v) for k, v in inputs.items()}
    for bname in ("gru_bih", "gru_bhh", "sfe_b", "fmean_b", "flogvar_b",
                  "lstm_bih", "lstm_bhh", "zmean_b", "zlogvar_b",
                  "p1_bih", "p1_bhh", "p2_bih", "p2_bhh", "pmean_b", "plogvar_b",
                  "decm_b", "decl_b", "enc_b"):
        assert not np.any(inp[bname]), f"nonzero bias {bname} unsupported"

    eps = _host_eps()
    nc = _get_program()

    f4 = np.float32
    bf = np.dtype("bfloat16") if hasattr(np, "bfloat16") else None
    import ml_dtypes
    bf = ml_dtypes.bfloat16
    shared = {
        "encw": _kt(inp["enc_w"].astype(f4), 8).astype(bf),
        "encb": np.ascontiguousarray(inp["enc_b"].astype(f4).reshape(4, 128).T),
        "gruwih": _kt(inp["gru_wih"].astype(f4), 4).astype(bf),
        "gruwhh": _kt(inp["gru_whh"].astype(f4), 4),
        "sfew": _kt(inp["sfe_w"].astype(f4), 4),
        "fcatw": _kt(np.concatenate(
            [inp["fmean_w"], inp["flogvar_w"]], axis=1).astype(f4), 4),
        "lstmwih": _kt(inp["lstm_wih"].astype(f4), 4).astype(bf),
        "lstmwhh": _kt(inp["lstm_whh"].astype(f4), 4).astype(bf),
        "zcatw": _kt(np.concatenate(
            [inp["zmean_w"], inp["zlogvar_w"]], axis=1).astype(f4), 4).astype(bf),
        "p1wih": np.ascontiguousarray(_permute_ifog(inp["p1_wih"].astype(f4))),
        "p1whh": _kt(_permute_ifog(inp["p1_whh"].astype(f4)), 2),
        "p2wih": _kt(_permute_ifog(inp["p2_wih"].astype(f4)), 2),
        "p2whh": _kt(_permute_ifog(inp["p2_whh"].astype(f4)), 2),
        "pcatw": _kt(np.concatenate(
            [inp["pmean_w"], inp["plogvar_w"]], axis=1).astype(f4), 2),
        "deccatw": _kt(
            np.concatenate([
                np.concatenate(
                    [inp["decm_w"], inp["decl_w"]], axis=1).astype(f4),
                np.zeros((384 - 288, 2 * FEAT), f4)], axis=0), 3),
    }

    in_maps = []
    for c in range(NCORE):
        bs = slice(c * BC, (c + 1) * BC)
        x = inp["x_seq"][bs].astype(f4).reshape(BT, FEAT)
        m = dict(shared)
        m["xT"] = np.ascontiguousarray(x.reshape(BT, 8, 128).transpose(2, 1, 0)).astype(bf)
        m["epsf"] = np.ascontiguousarray(eps["f"][bs, 0])
        m["ffT"] = np.ascontiguousarray(
            eps["ff"][bs].T.reshape(4, 128, BC).transpose(1, 0, 2)).astype(bf)
        m["epszT"] = np.ascontiguousarray(
            eps["z"][bs].transpose(2, 0, 1).reshape(ZD, RW))
        m["epsprT"] = np.ascontiguousarray(
            eps["pr"][:, bs].transpose(2, 0, 1).reshape(ZD, RW))
        epsd = np.concatenate(
            [eps["dx"][bs].reshape(BC * NW, FEAT).T,
             eps["ds"][bs, 0].T], axis=1)  # [1024, 272]
        m["epsdT"] = np.ascontiguousarray(
            epsd.reshape(8, 128, DKC).transpose(1, 0, 2))
        in_maps.append(m)

    res = run_bass_kernel_spmd(nc, in_maps, list(range(NCORE)))

    f_mean = np.empty((B, FD), f4)
    f_logvar = np.empty((B, FD), f4)
    f_post = np.empty((B, FD), f4)
    z_mean = np.empty((B, NW, ZD), f4)
    z_logvar = np.empty((B, NW, ZD), f4)
    z_post = np.empty((B, NW, ZD), f4)
    zp_mean = np.empty((B, NW, ZD), f4)
    zp_logvar = np.empty((B, NW, ZD), f4)
    z_prior = np.empty((B, NW, ZD), f4)
    recon_x = np.empty((B, NW, FEAT), f4)
    recon_x_frame = np.empty((B, 1, FEAT), f4)

    for c in range(NCORE):
        r = res.results[c]
        bs = slice(c * BC, (c + 1) * BC)
        f_mean[bs] = r["fml"][:, 0:FD]
        f_logvar[bs] = r["fml"][:, FD:2 * FD]
        f_post[bs] = r["fpost"]
        z_mean[bs] = r["zml"][0:ZD].reshape(ZD, BC, NW).transpose(1, 2, 0)
        z_logvar[bs] = r["zml"][ZD:2 * ZD].reshape(ZD, BC, NW).transpose(1, 2, 0)
        z_post[bs] = r["zpost"].reshape(ZD, BC, NW).transpose(1, 2, 0)
        zp_mean[bs] = r["pml"][0:ZD].reshape(ZD, NW, BC).transpose(2, 1, 0)
        zp_logvar[bs] = r["pml"][ZD:2 * ZD].reshape(ZD, NW, BC).transpose(2, 1, 0)
        z_prior[bs] = r["zprior"].reshape(ZD, NW, BC).transpose(2, 1, 0)
        rc = r["recon"].transpose(1, 0, 2).reshape(FEAT, DKC)
        recon_x[bs] = rc[:, 0:RW].reshape(FEAT, BC, NW).transpose(1, 2, 0)
        recon_x_frame[bs] = rc[:, RW:DKC].T[:, None, :]

    return (f_mean, f_logvar, f_post, z_mean, z_logvar, z_post,
            zp_mean, zp_logvar, z_prior, recon_x, recon_x_frame)


# revision 14
# speedup vs baseline: 1.2870x; 1.2870x over previous
"""Trainium2 Bass kernel for the DBSE sequential-VAE forward pass.

Data-parallel over batch B=128 across 8 NeuronCores (16 sequences/core).
All matmuls run as float32r (TF32) with fp32 PSUM accumulation.

Self-contained: hardcodes shapes; does not read sibling files.
"""

from contextlib import ExitStack

import numpy as np

import concourse.mybir as mybir
import concourse.tile as tile
from concourse import bacc
from concourse.bass_utils import run_bass_kernel_spmd
from concourse.masks import make_identity

f32 = mybir.dt.float32
f32r = mybir.dt.float32r
bf16 = mybir.dt.bfloat16
AF = mybir.ActivationFunctionType

B, T, FEAT, FC, WIN, ZD, FD, HID = 128, 128, 1024, 512, 8, 32, 256, 256
NW = T // WIN          # 16 windows
NCORE = 8
BC = B // NCORE        # 16 sequences per core
RW = BC * NW           # 256 dyn rows per core
BT = BC * T            # 2048 encoder rows per core
G4 = 4 * FC            # 2048 lstm gate width
G3 = 3 * FC            # 1536 gru gate width
DKC = 272              # decoder rhs cols: 256 (b,w) + 16 stat


def _build_program():
    nc = bacc.Bacc("TRN2", target_bir_lowering=False)

    d = {}

    def di(name, shape, dt=f32r):
        d[name] = nc.dram_tensor(name, shape, dt, kind="ExternalInput")

    def do(name, shape, dt=f32):
        d[name] = nc.dram_tensor(name, shape, dt, kind="ExternalOutput")

    di("xT", [128, 8, BT])            # x transposed, [p, k, (b t)]
    di("encw", [128, 8, FC])
    di("encb", [128, 4], f32)
    di("gruwih", [128, 4, G3])
    di("gruwhh", [128, 4, G3])
    di("sfew", [128, 4, FC])
    di("fcatw", [128, 4, FC])         # [fmean_w | flogvar_w]
    di("lstmwih", [128, 4, G4])
    di("lstmwhh", [128, 4, G4])
    di("zcatw", [128, 4, 2 * ZD])     # [zmean | zlogvar]
    di("p1wih", [ZD, 4 * HID])        # gate order (i,f,o,g)
    di("p1whh", [128, 2, 4 * HID])
    di("p2wih", [128, 2, 4 * HID])
    di("p2whh", [128, 2, 4 * HID])
    di("pcatw", [128, 2, 2 * ZD])     # [pmean | plogvar]
    di("deccatw", [128, 3, 2 * FEAT])  # [decm|decl], K padded 288->384

    di("epsf", [BC, FD], f32)
    di("ffT", [128, 4, BC], f32)
    di("epszT", [ZD, RW], f32)        # cols (b, w)
    di("epsprT", [ZD, RW], f32)       # cols (t, b)
    di("epsdT", [128, 8, DKC], f32)

    do("fml", [BC, 2 * FD])
    do("fpost", [BC, FD])
    do("zml", [2 * ZD, RW])
    do("zpost", [ZD, RW], f32r)
    do("pml", [2 * ZD, RW])
    do("zprior", [ZD, RW])
    do("recon", [128, 8, DKC])

    with tile.TileContext(nc) as tc:
        _emit(nc, tc, d)
    nc.finalize()
    return nc


def _emit(nc, tc, d):
    ctx = ExitStack()
    with ctx:
        const = ctx.enter_context(tc.tile_pool(name="const", bufs=1))
        state = ctx.enter_context(tc.tile_pool(name="state", bufs=1))
        dramp = ctx.enter_context(tc.tile_pool(name="dramp", bufs=1, space="DRAM"))

        ident = const.tile([128, 128], f32)
        make_identity(nc, ident)
        encb_t = const.tile([128, 4], f32)
        nc.sync.dma_start(encb_t[:], d["encb"][:])
        zeros_t = const.tile([128, 1024], f32)
        nc.vector.memset(zeros_t[:], 0.0)

        # PE warmup: dense dummy matmuls to engage HAM while first DMAs land
        with (
            tc.tile_pool(name="warmp", bufs=1) as warmp,
            tc.tile_pool(name="psW", bufs=1, space="PSUM") as psW,
        ):
            warm_rhs = warmp.tile([128, 256], f32r)
            nc.vector.tensor_copy(warm_rhs[:], zeros_t[:, 0:256])
            wps = psW.tile([128, 256], f32, tag="warm", bufs=1)
            for i in range(100):
                nc.tensor.matmul(wps[:], warm_rhs[:, 0:128], warm_rhs[:],
                                 start=(i == 0), stop=(i == 99))

        zpostT = state.tile([ZD, RW], f32r, tag="zpostT")
        fpostT = state.tile([128, 2, BC], f32r, tag="fpostT")
        zpv = zpostT.rearrange("p (b w) -> p b w", b=BC)
        gi_dram = dramp.tile([128, G3], f32)

        # =================== phases A (enc/GRU/f) + B (dyn/z) ===================
        with (
            tc.tile_pool(name="wlstm", bufs=1) as wlstm,
            tc.tile_pool(name="fcpool", bufs=1) as fcpool,
            tc.tile_pool(name="psGRU", bufs=1, space="PSUM") as psGRU,
        ):
            # lstm weight tiles; DMAs emitted after the encoder loop so the
            # encoder inputs win the DMA queues at startup
            lstmwih_t = wlstm.tile([128, 4, G4], f32r)
            lstmwhh_t = wlstm.tile([128, 4, G4], f32r)
            zcatw_t = wlstm.tile([128, 4, 2 * ZD], f32r)

            fc_T = fcpool.tile([128, 4, BT], f32r)   # tanh(enc) feature-major
            fcv = fc_T.rearrange("p k (b t) -> p k b t", b=BC)

            # ---------- encoder + gi precompute (scope frees before GRU) ----------
            with (
                tc.tile_pool(name="wENC", bufs=1) as wENC,
                tc.tile_pool(name="psP1", bufs=1, space="PSUM") as psP1,
            ):
                encw_t = wENC.tile([128, 8, FC], f32r)
                nc.sync.dma_start(encw_t[:], d["encw"][:])
                gruwih_t = wENC.tile([128, 4, G3], f32r)
                ffT_t = wENC.tile([128, 4, BC], f32)

                for c in range(4):
                    xt = wENC.tile([128, 8, 512], f32r, tag="x", bufs=2)
                    nc.sync.dma_start(
                        xt[:], d["xT"][:, :, c * 512:(c + 1) * 512])
                    for m in range(4):
                        ps = psP1.tile([128, 512], f32, tag="enc", bufs=2)
                        for k in range(8):
                            nc.tensor.matmul(
                                ps[:],
                                encw_t[:, k, m * 128:(m + 1) * 128],
                                xt[:, k, :],
                                start=(k == 0), stop=(k == 7),
                            )
                        nc.scalar.activation(
                            fc_T[:, m, c * 512:(c + 1) * 512], ps[:],
                            AF.Tanh, bias=encb_t[:, m:m + 1],
                        )

                nc.sync.dma_start(gruwih_t[:], d["gruwih"][:])
                nc.sync.dma_start(ffT_t[:], d["ffT"][:])
                nc.sync.dma_start(lstmwih_t[:], d["lstmwih"][:])
                nc.sync.dma_start(lstmwhh_t[:], d["lstmwhh"][:])
                nc.sync.dma_start(zcatw_t[:], d["zcatw"][:])

                # gi = x_win @ gru_wih, rows (t, b); staged via DRAM
                fcwin = wENC.tile([128, 4, 128], f32r)   # cols (t, b)
                nc.vector.tensor_copy(
                    fcwin.rearrange("p k (t b) -> p k t b", t=WIN),
                    fcv[:, :, :, 0:WIN].rearrange("p k b t -> p k t b"),
                )
                for n in range(3):
                    ps = psP1.tile([128, 512], f32, tag="enc", bufs=2)
                    for k in range(4):
                        nc.tensor.matmul(
                            ps[:], fcwin[:, k, :],
                            gruwih_t[:, k, n * 512:(n + 1) * 512],
                            start=(k == 0), stop=(k == 3),
                        )
                    gtmp = wENC.tile([128, 512], f32, tag="gtmp", bufs=2)
                    nc.vector.tensor_copy(gtmp[:], ps[:])
                    nc.sync.dma_start(gi_dram[:, n * 512:(n + 1) * 512], gtmp[:])

                # overwrite fc window-0 cols with ff (dyn branch input)
                nc.vector.tensor_copy(
                    fcv[:, :, :, 0:WIN],
                    ffT_t[:, :, :, None].to_broadcast([128, 4, BC, WIN]),
                )

            # ---------------- GRU + static feature chain + dyn ----------------
            with tc.tile_pool(name="wGRU", bufs=1) as wGRU:
                gruwhh_t = wGRU.tile([128, 4, G3], f32r)
                nc.sync.dma_start(gruwhh_t[:], d["gruwhh"][:])
                sfew_t = wGRU.tile([128, 4, FC], f32r)
                nc.sync.dma_start(sfew_t[:], d["sfew"][:])
                fcatw_t = wGRU.tile([128, 4, FC], f32r)
                nc.sync.dma_start(fcatw_t[:], d["fcatw"][:])
                epsf_t = wGRU.tile([BC, FD], f32)
                nc.sync.dma_start(epsf_t[:], d["epsf"][:])

                hB = wGRU.tile([BC, FC], f32, tag="gruh", bufs=2)
                nc.vector.memset(hB[:], 0.0)
                hT = wGRU.tile([128, 4, BC], f32r, tag="gruhT", bufs=2)
                nc.vector.tensor_copy(hT.rearrange("p k b -> p (k b)"), zeros_t[:, 0:64])
                for t in range(WIN):
                    gi_t = wGRU.tile([BC, G3], f32, tag="git", bufs=1)
                    nc.sync.dma_start(gi_t[:], gi_dram[t * BC:(t + 1) * BC, :])
                    pss = []
                    for n in range(3):
                        ps = psGRU.tile([BC, 512], f32, tag="gh", bufs=2)
                        for k in range(4):
                            nc.tensor.matmul(
                                ps[:], hT[:, k, :],
                                gruwhh_t[:, k, n * 512:(n + 1) * 512],
                                start=(k == 0), stop=(k == 3),
                            )
                        pss.append(ps)
                    prz = wGRU.tile([BC, 1024], f32, tag="prz", bufs=1)
                    nc.vector.tensor_add(prz[:, 0:512], gi_t[:, 0:512], pss[0][:])
                    nc.vector.tensor_add(
                        prz[:, 512:1024], gi_t[:, 512:1024], pss[1][:])
                    srz = wGRU.tile([BC, 1024], f32, tag="srz", bufs=1)
                    nc.scalar.activation(srz[:], prz[:], AF.Sigmoid)
                    rhn = wGRU.tile([BC, 512], f32, tag="rhn", bufs=1)
                    nc.vector.tensor_mul(rhn[:], srz[:, 0:512], pss[2][:])
                    nc.vector.tensor_add(rhn[:], rhn[:], gi_t[:, 1024:1536])
                    n_g = wGRU.tile([BC, 512], f32, tag="ng", bufs=1)
                    nc.scalar.activation(n_g[:], rhn[:], AF.Tanh)
                    dd = wGRU.tile([BC, 512], f32, tag="dd", bufs=1)
                    nc.vector.tensor_sub(dd[:], hB[:], n_g[:])
                    nc.vector.tensor_mul(dd[:], srz[:, 512:1024], dd[:])
                    hB = wGRU.tile([BC, FC], f32, tag="gruh", bufs=2)
                    nc.vector.tensor_add(hB[:], n_g[:], dd[:])
                    pst = psGRU.tile([128, 64], f32, tag="tp", bufs=1)
                    for j in range(4):
                        nc.tensor.transpose(
                            pst[:, j * BC:(j + 1) * BC],
                            hB[:, j * 128:(j + 1) * 128], ident[:BC, :BC],
                        )
                    hT = wGRU.tile([128, 4, BC], f32r, tag="gruhT", bufs=2)
                    nc.vector.tensor_copy(
                        hT[:], pst.rearrange("p (k b) -> p k b", k=4))

                # ---- static feature f, f_mean/f_logvar, f_post ----
                psf = psGRU.tile([BC, 512], f32, tag="gh", bufs=2)
                for k in range(4):
                    nc.tensor.matmul(psf[:], hT[:, k, :], sfew_t[:, k, :],
                                     start=(k == 0), stop=(k == 3))
                fB = wGRU.tile([BC, FC], f32, tag="fB")
                nc.scalar.activation(fB[:], psf[:], AF.Tanh)
                pstf = psGRU.tile([128, 64], f32, tag="tp", bufs=1)
                for j in range(4):
                    nc.tensor.transpose(
                        pstf[:, j * BC:(j + 1) * BC],
                        fB[:, j * 128:(j + 1) * 128], ident[:BC, :BC],
                    )
                fT = wGRU.tile([128, 4, BC], f32r, tag="fT")
                nc.vector.tensor_copy(
                    fT[:], pstf.rearrange("p (k b) -> p k b", k=4))
                psfm = psGRU.tile([BC, 512], f32, tag="gh", bufs=2)
                for k in range(4):
                    nc.tensor.matmul(psfm[:], fT[:, k, :], fcatw_t[:, k, :],
                                     start=(k == 0), stop=(k == 3))
                fml = wGRU.tile([BC, 512], f32, tag="fml")
                nc.vector.tensor_copy(fml[:], psfm[:])
                nc.sync.dma_start(d["fml"][:], fml[:])
                # f_post = mean + epsf * exp(0.5*logvar); exp via sigmoid ratio
                sa = wGRU.tile([BC, FD], f32, tag="sa")
                nc.scalar.activation(sa[:], fml[:, FD:2 * FD], AF.Sigmoid,
                                     scale=0.5)
                sb = wGRU.tile([BC, FD], f32, tag="sb")
                nc.scalar.activation(sb[:], fml[:, FD:2 * FD], AF.Sigmoid,
                                     scale=-0.5)
                nc.vector.reciprocal(sb[:], sb[:])
                nc.vector.tensor_mul(sa[:], sa[:], sb[:])       # exp(0.5 lv)
                nc.vector.tensor_mul(sa[:], sa[:], epsf_t[:])
                fpost = wGRU.tile([BC, FD], f32, tag="fpost")
                nc.vector.tensor_add(fpost[:], fml[:, 0:FD], sa[:])
                nc.sync.dma_start(d["fpost"][:], fpost[:])
                pstp = psGRU.tile([128, 64], f32, tag="tp", bufs=1)
                for j in range(2):
                    nc.tensor.transpose(
                        pstp[:, j * BC:(j + 1) * BC],
                        fpost[:, j * 128:(j + 1) * 128], ident[:BC, :BC],
                    )
                nc.vector.tensor_copy(
                    fpostT[:], pstp[:, 0:32].rearrange("p (k b) -> p k b", k=2))

                # =============== dynamic LSTM (feature-major, fused) ===========
                with (
                    tc.tile_pool(name="dynp", bufs=1) as dynp,
                    tc.tile_pool(name="psDYN", bufs=1, space="PSUM") as psDYN,
                ):
                    hTd = dynp.tile([128, 4, RW], f32r, tag="dynh", bufs=2)
                    nc.vector.tensor_copy(hTd.rearrange("p k n -> p (k n)"), zeros_t[:])
                    cd = dynp.tile([128, 4 * RW], f32, tag="dync", bufs=1)
                    nc.vector.memset(cd[:], 0.0)
                    fcz = fc_T.rearrange("p k (b w t) -> p k b w t", b=BC, w=NW)
                    gnames = ("si", "sf", "tg", "so")

                    for t in range(WIN):
                        newg = []
                        for g, func in enumerate(
                                (AF.Sigmoid, AF.Sigmoid, AF.Tanh, AF.Sigmoid)):
                            ps = psDYN.tile([128, 4 * RW], f32, tag="dyn",
                                            bufs=2)
                            for mi in range(4):
                                m = g * 4 + mi
                                for k in range(4):
                                    nc.tensor.matmul(
                                        ps[:, mi * RW:(mi + 1) * RW],
                                        lstmwih_t[:, k, m * 128:(m + 1) * 128],
                                        fcz[:, k, :, :, t],
                                        start=(k == 0), stop=False,
                                    )
                                for k in range(4):
                                    nc.tensor.matmul(
                                        ps[:, mi * RW:(mi + 1) * RW],
                                        lstmwhh_t[:, k, m * 128:(m + 1) * 128],
                                        hTd[:, k, :],
                                        start=False, stop=(k == 3),
                                    )
                            gt = dynp.tile([128, 4 * RW], f32, tag=gnames[g],
                                           bufs=1)
                            nc.scalar.activation(gt[:], ps[:], func)
                            newg.append(gt)
                        si, sf, tg, so = newg
                        tmp = dynp.tile([128, 4 * RW], f32, tag="dyntmp",
                                        bufs=1)
                        nc.vector.tensor_mul(tmp[:], si[:], tg[:])
                        nc.vector.tensor_mul(cd[:], sf[:], cd[:])
                        nc.vector.tensor_add(cd[:], cd[:], tmp[:])
                        thc = dynp.tile([128, 4 * RW], f32, tag="dyntmp",
                                        bufs=1)
                        nc.scalar.activation(thc[:], cd[:], AF.Tanh)
                        hTd = dynp.tile([128, 4, RW], f32r, tag="dynh", bufs=2)
                        nc.vector.tensor_mul(
                            hTd.rearrange("p k n -> p (k n)"), so[:], thc[:])

                    # ---- z projection + z_post ----
                    psz = psDYN.tile([2 * ZD, RW], f32, tag="z", bufs=1)
                    for k in range(4):
                        nc.tensor.matmul(psz[:], zcatw_t[:, k, :],
                                         hTd[:, k, :],
                                         start=(k == 0), stop=(k == 3))
                    zml = dynp.tile([2 * ZD, RW], f32, tag="si", bufs=1)
                    nc.vector.tensor_copy(zml[:], psz[:])
                    nc.sync.dma_start(d["zml"][:], zml[:])
                    za = dynp.tile([ZD, RW], f32, tag="sf", bufs=1)
                    nc.scalar.activation(za[:], zml[ZD:2 * ZD, :], AF.Sigmoid,
                                         scale=0.5)
                    zb = dynp.tile([ZD, RW], f32, tag="tg", bufs=1)
                    nc.scalar.activation(zb[:], zml[ZD:2 * ZD, :], AF.Sigmoid,
                                         scale=-0.5)
                    nc.vector.reciprocal(zb[:], zb[:])
                    nc.vector.tensor_mul(za[:], za[:], zb[:])
                    epszT_t = dynp.tile([ZD, RW], f32, tag="so", bufs=1)
                    nc.sync.dma_start(epszT_t[:], d["epszT"][:])
                    nc.vector.tensor_mul(za[:], za[:], epszT_t[:])
                    nc.vector.tensor_add(zpostT[:], zml[0:ZD, :], za[:])
                    nc.sync.dma_start(d["zpost"][:], zpostT[:])

        # ============== phase C: prior (2-layer LSTM) + decoder ==============
        with (
            tc.tile_pool(name="wC", bufs=1) as wC,
            tc.tile_pool(name="psPRI", bufs=1, space="PSUM") as psPRI,
        ):
            p1wih_t = wC.tile([ZD, 4 * HID], f32r)
            nc.sync.dma_start(p1wih_t[:], d["p1wih"][:])
            p1whh_t = wC.tile([128, 2, 4 * HID], f32r)
            nc.sync.dma_start(p1whh_t[:], d["p1whh"][:])
            p2wih_t = wC.tile([128, 2, 4 * HID], f32r)
            nc.sync.dma_start(p2wih_t[:], d["p2wih"][:])
            p2whh_t = wC.tile([128, 2, 4 * HID], f32r)
            nc.sync.dma_start(p2whh_t[:], d["p2whh"][:])
            pcatw_t = wC.tile([128, 2, 2 * ZD], f32r)
            nc.sync.dma_start(pcatw_t[:], d["pcatw"][:])
            deccatw_t = wC.tile([128, 3, 2 * FEAT], f32r)
            nc.sync.dma_start(deccatw_t[:], d["deccatw"][:])
            epsprT_t = wC.tile([ZD, RW], f32)
            nc.sync.dma_start(epsprT_t[:], d["epsprT"][:])
            epsdT_t = wC.tile([128, 8, DKC], f32)
            nc.sync.dma_start(epsdT_t[:], d["epsdT"][:])

            # zin: teacher-forced z_post shifted by one window, cols (t, b)
            zinT = wC.tile([ZD, RW], f32r)
            nc.vector.tensor_copy(zinT[:, 0:BC], zeros_t[0:32, 0:BC])
            nc.vector.tensor_copy(
                zinT.rearrange("p (t b) -> p t b", t=NW)[:, 1:NW, :],
                zpv[:, :, 0:NW - 1].rearrange("p b w -> p w b"),
            )

            h1T = wC.tile([128, 2, BC], f32r, tag="h1T", bufs=2)
            nc.vector.tensor_copy(h1T.rearrange("p k b -> p (k b)"), zeros_t[:, 0:32])
            c1 = wC.tile([BC, HID], f32, tag="c1", bufs=2)
            nc.vector.memset(c1[:], 0.0)
            c2 = wC.tile([BC, HID], f32, tag="c2", bufs=2)
            nc.vector.memset(c2[:], 0.0)
            h2all = wC.tile([128, 2, RW], f32r)
            h2z = wC.tile([128, 2, BC], f32r)
            nc.vector.tensor_copy(h2z.rearrange("p k b -> p (k b)"), zeros_t[:, 0:32])

            def lstm_cell(psg, cprev, ctag):
                """gates (i,f,o,g) psum [16,1024] -> (h [16,256], c_new)"""
                # tanh(g) first (ACT), then sig(i,f) so the c-chain starts asap;
                # sig(o) runs on ACT while DVE does the c update.
                tg = wC.tile([BC, HID], f32, tag="tg" + ctag, bufs=2)
                nc.scalar.activation(tg[:], psg[:, 3 * HID:4 * HID], AF.Tanh)
                sif = wC.tile([BC, 2 * HID], f32, tag="sif" + ctag, bufs=2)
                nc.scalar.activation(sif[:], psg[:, 0:2 * HID], AF.Sigmoid)
                so = wC.tile([BC, HID], f32, tag="so" + ctag, bufs=2)
                nc.scalar.activation(so[:], psg[:, 2 * HID:3 * HID], AF.Sigmoid)
                cn = wC.tile([BC, HID], f32, tag=ctag, bufs=2)
                nc.vector.tensor_mul(cn[:], sif[:, HID:2 * HID], cprev[:])
                tmp = wC.tile([BC, HID], f32, tag="tmp" + ctag, bufs=2)
                nc.vector.tensor_mul(tmp[:], sif[:, 0:HID], tg[:])
                nc.vector.tensor_add(cn[:], cn[:], tmp[:])
                th = wC.tile([BC, HID], f32, tag="th" + ctag, bufs=2)
                nc.scalar.activation(th[:], cn[:], AF.Tanh)
                hb = wC.tile([BC, HID], f32, tag="hb" + ctag, bufs=2)
                nc.vector.tensor_mul(hb[:], so[:], th[:])
                return hb, cn

            fillps = psPRI.tile([128, 256], f32, tag="fill", bufs=1)
            nc.tensor.matmul(fillps[:], zpostT[:, 0:128], zpostT[:],
                             start=True, stop=False)
            for i in range(60):
                nc.tensor.matmul(fillps[:], zpostT[:, 0:128], zpostT[:],
                                 start=False, stop=(i == 59))

            for t in range(NW):
                ps1 = psPRI.tile([BC, 4 * HID], f32, tag="pg", bufs=2)
                for n in range(2):
                    nc.tensor.matmul(
                        ps1[:, n * 512:(n + 1) * 512],
                        zinT[:, t * BC:(t + 1) * BC],
                        p1wih_t[:, n * 512:(n + 1) * 512],
                        start=True, stop=False,
                    )
                    for k in range(2):
                        nc.tensor.matmul(
                            ps1[:, n * 512:(n + 1) * 512],
                            h1T[:, k, :], p1whh_t[:, k, n * 512:(n + 1) * 512],
                            start=False, stop=(k == 1),
                        )
                h1b, c1 = lstm_cell(ps1, c1, "c1")
                pst1 = psPRI.tile([128, 32], f32, tag="ptp", bufs=1)
                for j in range(2):
                    nc.tensor.transpose(
                        pst1[:, j * BC:(j + 1) * BC],
                        h1b[:, j * 128:(j + 1) * 128], ident[:BC, :BC],
                    )
                h1T = wC.tile([128, 2, BC], f32r, tag="h1T", bufs=2)
                nc.vector.tensor_copy(
                    h1T[:], pst1.rearrange("p (k b) -> p k b", k=2))

                h2prev = h2z if t == 0 else h2all[:, :, (t - 1) * BC:t * BC]
                ps2 = psPRI.tile([BC, 4 * HID], f32, tag="pg", bufs=2)
                for n in range(2):
                    for k in range(2):
                        nc.tensor.matmul(
                            ps2[:, n * 512:(n + 1) * 512],
                            h1T[:, k, :], p2wih_t[:, k, n * 512:(n + 1) * 512],
                            start=(k == 0), stop=False,
                        )
                    for k in range(2):
                        nc.tensor.matmul(
                            ps2[:, n * 512:(n + 1) * 512],
                            h2prev[:, k, :],
                            p2whh_t[:, k, n * 512:(n + 1) * 512],
                            start=False, stop=(k == 1),
                        )
                h2b, c2 = lstm_cell(ps2, c2, "c2")
                pst2 = psPRI.tile([128, 32], f32, tag="ptp", bufs=1)
                for j in range(2):
                    nc.tensor.transpose(
                        pst2[:, j * BC:(j + 1) * BC],
                        h2b[:, j * 128:(j + 1) * 128], ident[:BC, :BC],
                    )
                nc.vector.tensor_copy(
                    h2all[:, :, t * BC:(t + 1) * BC],
                    pst2.rearrange("p (k b) -> p k b", k=2),
                )

            # pmean/plogvar batched over all steps
            psp = psPRI.tile([2 * ZD, RW], f32, tag="ptp", bufs=1)
            for k in range(2):
                nc.tensor.matmul(psp[:], pcatw_t[:, k, :], h2all[:, k, :],
                                 start=(k == 0), stop=(k == 1))
            pml = wC.tile([2 * ZD, RW], f32)
            nc.vector.tensor_copy(pml[:], psp[:])
            nc.sync.dma_start(d["pml"][:], pml[:])
            pe = wC.tile([ZD, RW], f32)
            nc.scalar.activation(pe[:], pml[ZD:2 * ZD, :], AF.Exp, scale=0.5)
            nc.vector.tensor_mul(pe[:], pe[:], epsprT_t[:])
            nc.vector.tensor_add(pe[:], pml[0:ZD, :], pe[:])
            nc.sync.dma_start(d["zprior"][:], pe[:])

            # ---- decoder ----
            zfT = wC.tile([128, 3, DKC], f32r)
            nc.vector.tensor_copy(zfT[0:32, 0, 0:RW], zpostT[:])
            nc.vector.tensor_copy(zfT[0:32, 0, RW:DKC], zpv[:, :, 0])

            def f_rows(dst, src):
                nc.vector.tensor_copy(
                    dst[:, 0:RW].rearrange("p (b w) -> p b w", b=BC),
                    src[:, :, None].to_broadcast(list(src.shape) + [NW]),
                )
                nc.vector.tensor_copy(dst[:, RW:DKC], src[:])

            # zf row 32+f <- f_post feature f, in 32-partition blocks
            for f0 in range(0, FD, 32):
                r = 32 + f0
                f_rows(zfT[r % 128:r % 128 + 32, r // 128, :],
                       fpostT[f0 % 128:f0 % 128 + 32, f0 // 128, :])

            recon = wC.tile([128, 8, DKC], f32)
            for j in range(8):
                psdm = psPRI.tile([128, DKC], f32, tag="dec", bufs=2)
                psdl = psPRI.tile([128, DKC], f32, tag="dec", bufs=2)
                for ps, m in ((psdm, j), (psdl, j + 8)):
                    for k in range(2):
                        nc.tensor.matmul(
                            ps[:], deccatw_t[:, k, m * 128:(m + 1) * 128],
                            zfT[:, k, :], start=(k == 0), stop=False,
                        )
                    nc.tensor.matmul(
                        ps[:], deccatw_t[0:32, 2, m * 128:(m + 1) * 128],
                        zfT[0:32, 2, :], start=False, stop=True,
                    )
                ee = wC.tile([128, DKC], f32, tag="dece", bufs=2)
                nc.scalar.activation(ee[:], psdl[:], AF.Exp, scale=0.5)
                nc.vector.tensor_mul(ee[:], ee[:], epsdT_t[:, j, :])
                nc.vector.tensor_add(recon[:, j, :], psdm[:], ee[:])
            nc.sync.dma_start(d["recon"][:], recon[:])


_PROG_CACHE = {}


def _get_program():
    if "nc" not in _PROG_CACHE:
        _PROG_CACHE["nc"] = _build_program()
    return _PROG_CACHE["nc"]


def _host_eps():
    if "eps" in _PROG_CACHE:
        return _PROG_CACHE["eps"]
    import jax
    import jax.numpy as jnp
    cpu = jax.local_devices(backend="cpu")[0]
    with jax.default_device(cpu):
        kf, kff, kz, kpr, kdx, kds = jax.random.split(jax.random.key(7), 6)
        eps = {
            "f": np.asarray(jax.random.normal(kf, (B, 1, FD), jnp.float32)),
            "ff": np.asarray(jax.random.normal(kff, (B, FC), jnp.float32)),
            "z": np.asarray(jax.random.normal(kz, (B, NW, ZD), jnp.float32)),
            "pr": np.asarray(jax.random.normal(kpr, (NW, B, ZD), jnp.float32)),
            "dx": np.asarray(jax.random.normal(kdx, (B, NW, FEAT), jnp.float32)),
            "ds": np.asarray(jax.random.normal(kds, (B, 1, FEAT), jnp.float32)),
        }
    _PROG_CACHE["eps"] = eps
    return eps


def _kt(w, kt):
    """[K, N] -> [128, K//128, N] partition-tiled, contiguous."""
    K, N = w.shape
    assert K == kt * 128
    return np.ascontiguousarray(w.reshape(kt, 128, N).transpose(1, 0, 2))


def _permute_ifog(w):
    """LSTM gate columns (i,f,g,o) -> (i,f,o,g). w: [K, 4H]"""
    K, G = w.shape
    H = G // 4
    return np.concatenate(
        [w[:, 0:2 * H], w[:, 3 * H:4 * H], w[:, 2 * H:3 * H]], axis=1)


def kernel(**inputs):
    inp = {k: np.asarray(v) for k, v in inputs.items()}
    for bname in ("gru_bih", "gru_bhh", "sfe_b", "fmean_b", "flogvar_b",
                  "lstm_bih", "lstm_bhh", "zmean_b", "zlogvar_b",
                  "p1_bih", "p1_bhh", "p2_bih", "p2_bhh", "pmean_b", "plogvar_b",
                  "decm_b", "decl_b", "enc_b"):
        assert not np.any(inp[bname]), f"nonzero bias {bname} unsupported"

    eps = _host_eps()
    nc = _get_program()

    f4 = np.float32
    bf = np.dtype("bfloat16") if hasattr(np, "bfloat16") else None
    import ml_dtypes
    bf = ml_dtypes.bfloat16
    shared = {
        "encw": _kt(inp["enc_w"].astype(f4), 8),
        "encb": np.ascontiguousarray(inp["enc_b"].astype(f4).reshape(4, 128).T),
        "gruwih": _kt(inp["gru_wih"].astype(f4), 4),
        "gruwhh": _kt(inp["gru_whh"].astype(f4), 4),
        "sfew": _kt(inp["sfe_w"].astype(f4), 4),
        "fcatw": _kt(np.concatenate(
            [inp["fmean_w"], inp["flogvar_w"]], axis=1).astype(f4), 4),
        "lstmwih": _kt(inp["lstm_wih"].astype(f4), 4),
        "lstmwhh": _kt(inp["lstm_whh"].astype(f4), 4),
        "zcatw": _kt(np.concatenate(
            [inp["zmean_w"], inp["zlogvar_w"]], axis=1).astype(f4), 4),
        "p1wih": np.ascontiguousarray(_permute_ifog(inp["p1_wih"].astype(f4))),
        "p1whh": _kt(_permute_ifog(inp["p1_whh"].astype(f4)), 2),
        "p2wih": _kt(_permute_ifog(inp["p2_wih"].astype(f4)), 2),
        "p2whh": _kt(_permute_ifog(inp["p2_whh"].astype(f4)), 2),
        "pcatw": _kt(np.concatenate(
            [inp["pmean_w"], inp["plogvar_w"]], axis=1).astype(f4), 2),
        "deccatw": _kt(
            np.concatenate([
                np.concatenate(
                    [inp["decm_w"], inp["decl_w"]], axis=1).astype(f4),
                np.zeros((384 - 288, 2 * FEAT), f4)], axis=0), 3),
    }

    in_maps = []
    for c in range(NCORE):
        bs = slice(c * BC, (c + 1) * BC)
        x = inp["x_seq"][bs].astype(f4).reshape(BT, FEAT)
        m = dict(shared)
        m["xT"] = np.ascontiguousarray(x.reshape(BT, 8, 128).transpose(2, 1, 0))
        m["epsf"] = np.ascontiguousarray(eps["f"][bs, 0])
        m["ffT"] = np.ascontiguousarray(
            eps["ff"][bs].T.reshape(4, 128, BC).transpose(1, 0, 2))
        m["epszT"] = np.ascontiguousarray(
            eps["z"][bs].transpose(2, 0, 1).reshape(ZD, RW))
        m["epsprT"] = np.ascontiguousarray(
            eps["pr"][:, bs].transpose(2, 0, 1).reshape(ZD, RW))
        epsd = np.concatenate(
            [eps["dx"][bs].reshape(BC * NW, FEAT).T,
             eps["ds"][bs, 0].T], axis=1)  # [1024, 272]
        m["epsdT"] = np.ascontiguousarray(
            epsd.reshape(8, 128, DKC).transpose(1, 0, 2))
        in_maps.append(m)

    res = run_bass_kernel_spmd(nc, in_maps, list(range(NCORE)))

    f_mean = np.empty((B, FD), f4)
    f_logvar = np.empty((B, FD), f4)
    f_post = np.empty((B, FD), f4)
    z_mean = np.empty((B, NW, ZD), f4)
    z_logvar = np.empty((B, NW, ZD), f4)
    z_post = np.empty((B, NW, ZD), f4)
    zp_mean = np.empty((B, NW, ZD), f4)
    zp_logvar = np.empty((B, NW, ZD), f4)
    z_prior = np.empty((B, NW, ZD), f4)
    recon_x = np.empty((B, NW, FEAT), f4)
    recon_x_frame = np.empty((B, 1, FEAT), f4)

    for c in range(NCORE):
        r = res.results[c]
        bs = slice(c * BC, (c + 1) * BC)
        f_mean[bs] = r["fml"][:, 0:FD]
        f_logvar[bs] = r["fml"][:, FD:2 * FD]
        f_post[bs] = r["fpost"]
        z_mean[bs] = r["zml"][0:ZD].reshape(ZD, BC, NW).transpose(1, 2, 0)
        z_logvar[bs] = r["zml"][ZD:2 * ZD].reshape(ZD, BC, NW).transpose(1, 2, 0)
        z_post[bs] = r["zpost"].reshape(ZD, BC, NW).transpose(1, 2, 0)
        zp_mean[bs] = r["pml"][0:ZD].reshape(ZD, NW, BC).transpose(2, 1, 0)
        zp_logvar[bs] = r["pml"][ZD:2 * ZD].reshape(ZD, NW, BC).transpose(2, 1, 0)
        z_prior[bs] = r["zprior"].reshape(ZD, NW, BC).transpose(2, 1, 0)
        rc = r["recon"].transpose(1, 0, 2).reshape(FEAT, DKC)
        recon_x[bs] = rc[:, 0:RW].reshape(FEAT, BC, NW).transpose(1, 2, 0)
        recon_x_frame[bs] = rc[:, RW:DKC].T[:, None, :]

    return (f_mean, f_logvar, f_post, z_mean, z_logvar, z_post,
            zp_mean, zp_logvar, z_prior, recon_x, recon_x_frame)
